# revision 3
# baseline (speedup 1.0000x reference)
"""Antialiased 2x upsampling (StyleGAN2 upsample_2d, k=[1,3,3,1], factor=2).

Input  x: (8, 256, 256, 64) f32 NHWC  ->  output: (8, 511, 511, 64) f32.

Math (separable, polyphase; all taps exact in f32 up to one 1/3 rounding):
  A[i] = x[i-1] (zero-padded), B[i] = x[i]
  g = A/3 + B        (even out rows 2i,   scale deficit absorbed below)
  h = B/3 + A        (odd  out rows 2i-1)
  out[2i,   2j]   = 9/16*g[j]   + 3/16*g[j-1]
  out[2i,   2j-1] = 9/16*g[j-1] + 3/16*g[j]
  out[2i-1, 2j]   = 9/16*h[j]   + 3/16*h[j-1]
  out[2i-1, 2j-1] = 9/16*h[j-1] + 3/16*h[j]

Sharding: pure data parallel, one batch image per NeuronCore (8 cores).
Layout: partition dim = input row i (128 rows per h-tile), free dim = w*C+c.
All shifts are free-dim AP offsets except the H-shift, which is realized by
loading a second, row-shifted copy (A) of the input tile from DRAM.
"""

import numpy as np

import concourse.bacc as bacc
import concourse.mybir as mybir
from concourse.tile import TileContext
from concourse.bass_utils import run_bass_kernel_spmd

F32 = mybir.dt.float32
MULT = mybir.AluOpType.mult
ADD = mybir.AluOpType.add

B_FULL, H_FULL, W_FULL, C_FULL = 8, 256, 256, 64
N_CORES = 8


def build_upsample_tile(tc, out, x, H, W, C, P, WT):
    """Trace the upsampling kernel into TileContext tc.

    x:   DRAM AP [H, W*C]
    out: DRAM AP [2H-1, (2W-1)*C]
    P:   partition tile height (input rows per tile)
    WT:  input cols per w-tile
    """
    nc = tc.nc
    assert W % WT == 0
    n_wt = W // WT
    FW = (WT + 1) * C  # tile free width: cols w0-1 .. w0+WT-1

    # h-tiles cover input rows i = i0 .. i0+PT-1 (partition p <-> i = i0+p).
    # Row i produces out rows 2i (even) and 2i-1 (odd). i=0 is handled by a
    # separate 1-partition pass (out row 0 only) so that the shifted A-load
    # (src rows i0-1..) never reads row -1 and all SBUF APs start at
    # partition 0 (hardware allows starts only at 0/32/64/96).
    h_tiles = []
    i0 = 1
    while i0 < H:
        h_tiles.append((i0, min(P, H - i0)))
        i0 += P

    seg = 2 * WT * C

    with (
        tc.tile_pool(name="io", bufs=2) as io_pool,
        tc.tile_pool(name="mid", bufs=2) as mid_pool,
        tc.tile_pool(name="rb", bufs=2) as rb_pool,
    ):
        def v(t, qlo, PT):
            return t[:PT, qlo * C : (qlo + WT) * C].rearrange("p (j c) -> p j c", c=C)

        def wpass(f, f3, rbv, s, PT):
            # out[r, 2j]   = 9/16 f[j]   + f3[j-1]   (even cols -> q=1 slot)
            nc.vector.scalar_tensor_tensor(
                out=rbv[:PT, s, :, 1, :], in0=v(f, 1, PT), scalar=9.0 / 16.0,
                in1=v(f3, 0, PT), op0=MULT, op1=ADD,
            )
            # out[r, 2j-1] = 9/16 f[j-1] + f3[j]     (odd cols -> q=0 slot)
            nc.vector.scalar_tensor_tensor(
                out=rbv[:PT, s, :, 0, :], in0=v(f, 0, PT), scalar=9.0 / 16.0,
                in1=v(f3, 1, PT), op0=MULT, op1=ADD,
            )

        for wt in range(n_wt):
            w0 = wt * WT
            cl = (w0 - 1) * C  # DRAM col offset of tile col q=0
            skip = C if w0 == 0 else 0
            dcol_lo = 0 if w0 == 0 else (2 * w0 - 1) * C
            dcol_hi = dcol_lo + seg - skip

            # --- row 0 pass: out[0] = W-upsample of x[0] (A = x[-1] = 0) ---
            B0 = io_pool.tile([1, FW], F32, tag="B", name=f"B0_{wt}")
            if w0 == 0:
                nc.gpsimd.memset(B0[:, 0:C], 0.0)
                nc.sync.dma_start(out=B0[:, C:], in_=x[0:1, 0 : WT * C])
            else:
                nc.sync.dma_start(out=B0[:], in_=x[0:1, cl : cl + FW])
            g30 = mid_pool.tile([1, FW], F32, tag="g3", name=f"g30_{wt}")
            nc.scalar.mul(g30[:], B0[:], 3.0 / 16.0)
            rb0 = rb_pool.tile([1, 4 * WT * C], F32, tag="rb", name=f"rb0_{wt}")
            rbv0 = rb0.rearrange("p (s j q c) -> p s j q c", s=2, j=WT, q=2, c=C)
            wpass(B0, g30, rbv0, 0, 1)
            nc.sync.dma_start(out=out[0:1, dcol_lo:dcol_hi], in_=rb0[:1, skip:seg])

            # --- main h-tiles ---
            for ti, (i0, PT) in enumerate(h_tiles):
                A = io_pool.tile([PT, FW], F32, tag="A", name=f"A_{ti}_{wt}")
                Bt = io_pool.tile([PT, FW], F32, tag="B", name=f"B_{ti}_{wt}")
                # B[p] = x[i0+p], A[p] = x[i0+p-1]; cols w0-1 .. w0+WT-1
                if w0 == 0:
                    nc.gpsimd.memset(Bt[:, 0:C], 0.0)
                    nc.gpsimd.memset(A[:, 0:C], 0.0)
                    nc.sync.dma_start(out=Bt[:, C:], in_=x[i0 : i0 + PT, 0 : WT * C])
                    nc.sync.dma_start(
                        out=A[:, C:], in_=x[i0 - 1 : i0 + PT - 1, 0 : WT * C]
                    )
                else:
                    nc.sync.dma_start(out=Bt[:], in_=x[i0 : i0 + PT, cl : cl + FW])
                    nc.sync.dma_start(
                        out=A[:], in_=x[i0 - 1 : i0 + PT - 1, cl : cl + FW]
                    )

                g = mid_pool.tile([PT, FW], F32, tag="g", name=f"g_{ti}_{wt}")
                hh = mid_pool.tile([PT, FW], F32, tag="h", name=f"h_{ti}_{wt}")
                g3 = mid_pool.tile([PT, FW], F32, tag="g3", name=f"g3_{ti}_{wt}")
                h3 = mid_pool.tile([PT, FW], F32, tag="h3", name=f"h3_{ti}_{wt}")
                nc.vector.scalar_tensor_tensor(
                    out=g[:], in0=A[:], scalar=1.0 / 3.0, in1=Bt[:], op0=MULT, op1=ADD
                )
                nc.vector.scalar_tensor_tensor(
                    out=hh[:], in0=Bt[:], scalar=1.0 / 3.0, in1=A[:], op0=MULT, op1=ADD
                )
                nc.scalar.mul(g3[:], g[:], 3.0 / 16.0)
                nc.scalar.mul(h3[:], hh[:], 3.0 / 16.0)

                # rowbuf: [even-row seg | odd-row seg], seg = WT x [oddcol|evencol] x C
                rb = rb_pool.tile([PT, 4 * WT * C], F32, tag="rb", name=f"rb_{ti}_{wt}")
                rbv = rb.rearrange("p (s j q c) -> p s j q c", s=2, j=WT, q=2, c=C)
                wpass(g, g3, rbv, 0, PT)   # even rows 2i
                wpass(hh, h3, rbv, 1, PT)  # odd rows 2i-1

                # even rows 2(i0+p): 2*i0, 2*i0+2, ...
                nc.sync.dma_start(
                    out=out[2 * i0 : 2 * i0 + 2 * PT - 1 : 2, dcol_lo:dcol_hi],
                    in_=rb[:PT, skip:seg],
                )
                # odd rows 2(i0+p)-1
                nc.sync.dma_start(
                    out=out[2 * i0 - 1 : 2 * i0 + 2 * PT - 2 : 2, dcol_lo:dcol_hi],
                    in_=rb[:PT, seg + skip : 2 * seg],
                )


def build_nc(H=H_FULL, W=W_FULL, C=C_FULL, P=128, WT=32):
    nc = bacc.Bacc("TRN2", target_bir_lowering=False, debug=False)
    x = nc.declare_dram_parameter("x", [H, W * C], F32, isOutput=False).ap()
    out = nc.declare_dram_parameter(
        "out", [2 * H - 1, (2 * W - 1) * C], F32, isOutput=True
    ).ap()
    with TileContext(nc) as tc:
        build_upsample_tile(tc, out, x, H, W, C, P, WT)
    nc.compile()
    return nc


_NC_CACHE = {}


def _get_nc():
    key = (H_FULL, W_FULL, C_FULL)
    if key not in _NC_CACHE:
        _NC_CACHE[key] = build_nc()
    return _NC_CACHE[key]


def run_spmd(x, trace=False, **kwargs):
    """x: (8, 256, 256, 64) f32. Returns (BassKernelResults, out (8,511,511,64))."""
    nc = _get_nc()
    in_maps = [
        {"x": np.ascontiguousarray(x[b]).reshape(H_FULL, W_FULL * C_FULL)}
        for b in range(N_CORES)
    ]
    res = run_bass_kernel_spmd(
        nc, in_maps, core_ids=list(range(N_CORES)), trace=trace, **kwargs
    )
    out = np.stack(
        [
            res.results[b]["out"].reshape(2 * H_FULL - 1, 2 * W_FULL - 1, C_FULL)
            for b in range(N_CORES)
        ]
    )
    return res, out


def kernel(x):
    x = np.asarray(x, dtype=np.float32)
    _, out = run_spmd(x, trace=False)
    return out


# revision 4
# speedup vs baseline: 2.0209x; 2.0209x over previous
"""Antialiased 2x upsampling (StyleGAN2 upsample_2d, k=[1,3,3,1], factor=2).

Input  x: (8, 256, 256, 64) f32 NHWC  ->  output: (8, 511, 511, 64) f32.

Math (separable, polyphase; all taps exact in f32 up to one 1/3 rounding):
  A[i] = x[i-1] (zero-padded), B[i] = x[i]
  g = A/3 + B        (even out rows 2i,   scale deficit absorbed below)
  h = B/3 + A        (odd  out rows 2i-1)
  out[2i,   2j]   = 9/16*g[j]   + 3/16*g[j-1]
  out[2i,   2j-1] = 9/16*g[j-1] + 3/16*g[j]
  out[2i-1, 2j]   = 9/16*h[j]   + 3/16*h[j-1]
  out[2i-1, 2j-1] = 9/16*h[j-1] + 3/16*h[j]

Sharding: pure data parallel, one batch image per NeuronCore (8 cores).
Layout: partition dim = input row i (128 rows per h-tile), free dim = w*C+c.
All shifts are free-dim AP offsets except the H-shift, which is realized by
loading a second, row-shifted copy (A) of the input tile from DRAM.
"""

import numpy as np

import concourse.bacc as bacc
import concourse.mybir as mybir
from concourse.tile import TileContext
from concourse.bass_utils import run_bass_kernel_spmd

F32 = mybir.dt.float32
MULT = mybir.AluOpType.mult
ADD = mybir.AluOpType.add

B_FULL, H_FULL, W_FULL, C_FULL = 8, 256, 256, 64
N_CORES = 8


def build_upsample_tile(tc, out, x, H, W, C, P, WT):
    """Trace the upsampling kernel into TileContext tc.

    x:   DRAM AP [H, W*C]
    out: DRAM AP [2H-1, (2W-1)*C]
    P:   partition tile height (input rows per tile)
    WT:  input cols per w-tile
    """
    nc = tc.nc
    assert W % WT == 0
    n_wt = W // WT
    FW = (WT + 1) * C  # tile free width: cols w0-1 .. w0+WT-1

    # h-tiles cover input rows i = i0 .. i0+PT-1 (partition p <-> i = i0+p).
    # Row i produces out rows 2i (even) and 2i-1 (odd). i=0 is handled by a
    # separate 1-partition pass (out row 0 only) so that the shifted A-load
    # (src rows i0-1..) never reads row -1 and all SBUF APs start at
    # partition 0 (hardware allows starts only at 0/32/64/96).
    h_tiles = []
    i0 = 1
    while i0 < H:
        h_tiles.append((i0, min(P, H - i0)))
        i0 += P

    seg = 2 * WT * C

    with (
        tc.tile_pool(name="io", bufs=2) as io_pool,
        tc.tile_pool(name="mid", bufs=2) as mid_pool,
        tc.tile_pool(name="rb", bufs=2) as rb_pool,
    ):
        def v(t, qlo, PT):
            return t[:PT, qlo * C : (qlo + WT) * C].rearrange("p (j c) -> p j c", c=C)

        def wpass(f, f3, rbv, s, PT):
            # out[r, 2j]   = 9/16 f[j]   + f3[j-1]   (even cols -> q=1 slot)
            nc.vector.scalar_tensor_tensor(
                out=rbv[:PT, s, :, 1, :], in0=v(f, 1, PT), scalar=9.0 / 16.0,
                in1=v(f3, 0, PT), op0=MULT, op1=ADD,
            )
            # out[r, 2j-1] = 9/16 f[j-1] + f3[j]     (odd cols -> q=0 slot)
            nc.vector.scalar_tensor_tensor(
                out=rbv[:PT, s, :, 0, :], in0=v(f, 0, PT), scalar=9.0 / 16.0,
                in1=v(f3, 1, PT), op0=MULT, op1=ADD,
            )

        for wt in range(n_wt):
            w0 = wt * WT
            cl = (w0 - 1) * C  # DRAM col offset of tile col q=0
            skip = C if w0 == 0 else 0
            dcol_lo = 0 if w0 == 0 else (2 * w0 - 1) * C
            dcol_hi = dcol_lo + seg - skip

            # --- row 0 pass: out[0] = W-upsample of x[0] (A = x[-1] = 0) ---
            B0 = io_pool.tile([1, FW], F32, tag="B", name=f"B0_{wt}")
            if w0 == 0:
                nc.gpsimd.memset(B0[:, 0:C], 0.0)
                nc.gpsimd.dma_start(out=B0[:, C:], in_=x[0:1, 0 : WT * C])
            else:
                nc.gpsimd.dma_start(out=B0[:], in_=x[0:1, cl : cl + FW])
            g30 = mid_pool.tile([1, FW], F32, tag="g3", name=f"g30_{wt}")
            nc.scalar.mul(g30[:], B0[:], 3.0 / 16.0)
            rb0 = rb_pool.tile([1, 4 * WT * C], F32, tag="rb", name=f"rb0_{wt}")
            rbv0 = rb0.rearrange("p (s j q c) -> p s j q c", s=2, j=WT, q=2, c=C)
            wpass(B0, g30, rbv0, 0, 1)
            nc.gpsimd.dma_start(out=out[0:1, dcol_lo:dcol_hi], in_=rb0[:1, skip:seg])

            # --- main h-tiles ---
            for ti, (i0, PT) in enumerate(h_tiles):
                A = io_pool.tile([PT, FW], F32, tag="A", name=f"A_{ti}_{wt}")
                Bt = io_pool.tile([PT, FW], F32, tag="B", name=f"B_{ti}_{wt}")
                # B[p] = x[i0+p], A[p] = x[i0+p-1]; cols w0-1 .. w0+WT-1
                if w0 == 0:
                    nc.gpsimd.memset(Bt[:, 0:C], 0.0)
                    nc.gpsimd.memset(A[:, 0:C], 0.0)
                    nc.gpsimd.dma_start(out=Bt[:, C:], in_=x[i0 : i0 + PT, 0 : WT * C])
                    nc.gpsimd.dma_start(
                        out=A[:, C:], in_=x[i0 - 1 : i0 + PT - 1, 0 : WT * C]
                    )
                else:
                    nc.gpsimd.dma_start(out=Bt[:], in_=x[i0 : i0 + PT, cl : cl + FW])
                    nc.gpsimd.dma_start(
                        out=A[:], in_=x[i0 - 1 : i0 + PT - 1, cl : cl + FW]
                    )

                g = mid_pool.tile([PT, FW], F32, tag="g", name=f"g_{ti}_{wt}")
                hh = mid_pool.tile([PT, FW], F32, tag="h", name=f"h_{ti}_{wt}")
                g3 = mid_pool.tile([PT, FW], F32, tag="g3", name=f"g3_{ti}_{wt}")
                h3 = mid_pool.tile([PT, FW], F32, tag="h3", name=f"h3_{ti}_{wt}")
                nc.vector.scalar_tensor_tensor(
                    out=g[:], in0=A[:], scalar=1.0 / 3.0, in1=Bt[:], op0=MULT, op1=ADD
                )
                nc.vector.scalar_tensor_tensor(
                    out=hh[:], in0=Bt[:], scalar=1.0 / 3.0, in1=A[:], op0=MULT, op1=ADD
                )
                nc.scalar.mul(g3[:], g[:], 3.0 / 16.0)
                nc.scalar.mul(h3[:], hh[:], 3.0 / 16.0)

                # rowbuf: [even-row seg | odd-row seg], seg = WT x [oddcol|evencol] x C
                rb = rb_pool.tile([PT, 4 * WT * C], F32, tag="rb", name=f"rb_{ti}_{wt}")
                rbv = rb.rearrange("p (s j q c) -> p s j q c", s=2, j=WT, q=2, c=C)
                wpass(g, g3, rbv, 0, PT)   # even rows 2i
                wpass(hh, h3, rbv, 1, PT)  # odd rows 2i-1

                # even rows 2(i0+p): 2*i0, 2*i0+2, ...
                nc.gpsimd.dma_start(
                    out=out[2 * i0 : 2 * i0 + 2 * PT - 1 : 2, dcol_lo:dcol_hi],
                    in_=rb[:PT, skip:seg],
                )
                # odd rows 2(i0+p)-1
                nc.gpsimd.dma_start(
                    out=out[2 * i0 - 1 : 2 * i0 + 2 * PT - 2 : 2, dcol_lo:dcol_hi],
                    in_=rb[:PT, seg + skip : 2 * seg],
                )


def build_nc(H=H_FULL, W=W_FULL, C=C_FULL, P=128, WT=32):
    nc = bacc.Bacc("TRN2", target_bir_lowering=False, debug=False)
    x = nc.declare_dram_parameter("x", [H, W * C], F32, isOutput=False).ap()
    out = nc.declare_dram_parameter(
        "out", [2 * H - 1, (2 * W - 1) * C], F32, isOutput=True
    ).ap()
    with TileContext(nc) as tc:
        build_upsample_tile(tc, out, x, H, W, C, P, WT)
    nc.compile()
    return nc


_NC_CACHE = {}


def _get_nc():
    key = (H_FULL, W_FULL, C_FULL)
    if key not in _NC_CACHE:
        _NC_CACHE[key] = build_nc()
    return _NC_CACHE[key]


def run_spmd(x, trace=False, **kwargs):
    """x: (8, 256, 256, 64) f32. Returns (BassKernelResults, out (8,511,511,64))."""
    nc = _get_nc()
    in_maps = [
        {"x": np.ascontiguousarray(x[b]).reshape(H_FULL, W_FULL * C_FULL)}
        for b in range(N_CORES)
    ]
    res = run_bass_kernel_spmd(
        nc, in_maps, core_ids=list(range(N_CORES)), trace=trace, **kwargs
    )
    out = np.stack(
        [
            res.results[b]["out"].reshape(2 * H_FULL - 1, 2 * W_FULL - 1, C_FULL)
            for b in range(N_CORES)
        ]
    )
    return res, out


def kernel(x):
    x = np.asarray(x, dtype=np.float32)
    _, out = run_spmd(x, trace=False)
    return out
